# revision 7
# baseline (speedup 1.0000x reference)
"""Trainium2 Bass kernel for DetectionLoss (L1 + GIoU + CAM supervision).

Contract: kernel(**inputs) takes the FULL unsharded inputs (numpy arrays,
keyed as in setup_inputs()) and returns the FULL output, a float32 [4] array
[loss_l1, loss_giou, loss_cam, loss_total].

Sharding: data-parallel over batch B=16 across 8 NeuronCores (2 batches per
core). Positives (pos_*) are routed host-side to the core that owns their
batch. Each core computes partial sums (l1, giou*w, cam-term) on device; the
host adds the 8 partial vectors and applies the final scaling.
"""

import numpy as np

# Problem constants (hardcoded per the task contract).
B, C, H, W, K = 16, 80, 64, 64, 32
NCORES = 8
BPC = B // NCORES           # batches per core = 2
PAIRS = BPC * K             # CAM (box,channel) pairs per core = 64
HW = H * W                  # 4096
HALF = HW // 2              # 2048 (channel map split into 2 partitions)
LAMBDA_L1, LAMBDA_GIOU, LAMBDA_CAM = 1.0, 2.0, 0.5
EPS = 1e-6

# fpack column layout (single packed f32 [128, FCOLS] input per core)
_KV0 = 0            # kvals: 1..63            -> cols [0, 63)
_CV0 = 63           # colvals: 0..63          -> cols [63, 127)
_RV0 = 127          # rowvals (per half)      -> cols [127, 159)
_GT0 = 159          # gt box coords (dup)     -> cols [159, 163)
_GL = 163           # gt label (dup, f32)
_BI = 164           # cam gather base index (f32)
_PW = 165           # positive slot weight
_PB = 166           # positive local batch (f32)
_PC = 167           # positive class (f32)
_PI = 168           # positive i (f32)
_PJ = 169           # positive j (f32)
_PG = 170           # positive gt index (f32)
_HT = 171           # half tag: 1.0 for partitions < 64, else 0.0
FCOLS = 172


def _build_kernel(debug=False):
    import concourse.bacc as bacc
    import concourse.mybir as mybir
    from concourse import bass
    from concourse.tile import TileContext

    f32 = mybir.dt.float32
    i32 = mybir.dt.int32
    Alu = mybir.AluOpType
    Act = mybir.ActivationFunctionType

    nc = bacc.Bacc("TRN2", target_bir_lowering=False, debug=False,
                   num_devices=NCORES)

    cam2 = nc.dram_tensor("cam2", [BPC * C * 2, HALF], f32, kind="ExternalInput")
    pred = nc.dram_tensor("pred", [BPC * C * H * W, 4], f32, kind="ExternalInput")
    gtb = nc.dram_tensor("gtb", [PAIRS, 4], f32, kind="ExternalInput")
    fpk = nc.dram_tensor("fpk", [128, FCOLS], f32, kind="ExternalInput")
    out = nc.dram_tensor("out", [4, 1], f32, kind="ExternalOutput")
    if debug:
        dbg = nc.dram_tensor("dbg", [128, 48], f32, kind="ExternalOutput")

    with TileContext(nc) as tc:
        with (
            tc.tile_pool(name="pool", bufs=1) as pool,
            tc.tile_pool(name="psum", bufs=1, space="PSUM") as pp,
        ):
            F = pool.tile([128, FCOLS], f32)
            nc.sync.dma_start(out=F[:], in_=fpk.ap())

            # ---- cam channel gather (the long pole: 128 x 8KB rows) ----
            CIF = pool.tile([128, 1], f32)
            nc.vector.scalar_tensor_tensor(
                out=CIF[:], in0=F[:, _GL:_GL + 1], scalar=2.0,
                in1=F[:, _BI:_BI + 1], op0=Alu.mult, op1=Alu.add)
            CI = pool.tile([128, 1], i32)
            nc.vector.tensor_copy(out=CI[:], in_=CIF[:])
            CAM = pool.tile([128, HALF], f32)
            nc.gpsimd.indirect_dma_start(
                out=CAM[:], out_offset=None, in_=cam2.ap(),
                in_offset=bass.IndirectOffsetOnAxis(ap=CI[:, :1], axis=0))

            # ---- positive box gathers ----
            X1 = pool.tile([128, 1], f32)
            nc.vector.scalar_tensor_tensor(
                out=X1[:], in0=F[:, _PB:_PB + 1], scalar=float(C),
                in1=F[:, _PC:_PC + 1], op0=Alu.mult, op1=Alu.add)
            X2 = pool.tile([128, 1], f32)
            nc.vector.scalar_tensor_tensor(
                out=X2[:], in0=X1[:], scalar=float(H),
                in1=F[:, _PI:_PI + 1], op0=Alu.mult, op1=Alu.add)
            X3 = pool.tile([128, 1], f32)
            nc.vector.scalar_tensor_tensor(
                out=X3[:], in0=X2[:], scalar=float(W),
                in1=F[:, _PJ:_PJ + 1], op0=Alu.mult, op1=Alu.add)
            X3I = pool.tile([128, 1], i32)
            nc.vector.tensor_copy(out=X3I[:], in_=X3[:])
            PBOX = pool.tile([128, 4], f32)
            nc.gpsimd.indirect_dma_start(
                out=PBOX[:], out_offset=None, in_=pred.ap(),
                in_offset=bass.IndirectOffsetOnAxis(ap=X3I[:, :1], axis=0))

            GIF = pool.tile([128, 1], f32)
            nc.vector.scalar_tensor_tensor(
                out=GIF[:], in0=F[:, _PB:_PB + 1], scalar=float(K),
                in1=F[:, _PG:_PG + 1], op0=Alu.mult, op1=Alu.add)
            GII = pool.tile([128, 1], i32)
            nc.vector.tensor_copy(out=GII[:], in_=GIF[:])
            GBOX = pool.tile([128, 4], f32)
            nc.gpsimd.indirect_dma_start(
                out=GBOX[:], out_offset=None, in_=gtb.ap(),
                in_offset=bass.IndirectOffsetOnAxis(ap=GII[:, :1], axis=0))

            # ---- partial-sum accumulator tile ----
            PART = pool.tile([128, 4], f32)
            nc.gpsimd.memset(PART[:], 0.0)
            ONES = pool.tile([128, 1], f32)
            nc.gpsimd.memset(ONES[:], 1.0)

            # ---- exact floor of 64*coords via comparison-sum ----
            SC = pool.tile([128, 4], f32)
            nc.vector.tensor_scalar_mul(SC[:], F[:, _GT0:_GT0 + 4], float(W))
            GEB = pool.tile([128, 4 * 63], f32)
            nc.vector.tensor_tensor(
                out=GEB[:].rearrange("p (c k) -> p c k", k=63),
                in0=SC[:].unsqueeze(2).to_broadcast([128, 4, 63]),
                in1=F[:, _KV0:_KV0 + 63].unsqueeze(1).to_broadcast([128, 4, 63]),
                op=Alu.is_ge)
            IC = pool.tile([128, 4], f32)  # jmin, imin, jmax, imax
            nc.vector.tensor_reduce(
                out=IC[:], in_=GEB[:].rearrange("p (c k) -> p c k", k=63),
                axis=mybir.AxisListType.X, op=Alu.add)

            # ---- row/col interval masks ----
            CGE = pool.tile([128, 64], f32)
            nc.vector.tensor_scalar(
                out=CGE[:], in0=F[:, _CV0:_CV0 + 64], scalar1=IC[:, 0:1],
                scalar2=None, op0=Alu.is_ge)
            CM = pool.tile([128, 64], f32)
            nc.vector.scalar_tensor_tensor(
                out=CM[:], in0=F[:, _CV0:_CV0 + 64], scalar=IC[:, 2:3],
                in1=CGE[:], op0=Alu.is_le, op1=Alu.mult)
            RGE = pool.tile([128, 32], f32)
            nc.vector.tensor_scalar(
                out=RGE[:], in0=F[:, _RV0:_RV0 + 32], scalar1=IC[:, 1:2],
                scalar2=None, op0=Alu.is_ge)
            RM = pool.tile([128, 32], f32)
            nc.vector.scalar_tensor_tensor(
                out=RM[:], in0=F[:, _RV0:_RV0 + 32], scalar=IC[:, 3:4],
                in1=RGE[:], op0=Alu.is_le, op1=Alu.mult)

            # ---- big ops on the gathered CAM halves ----
            ST = pool.tile([128, 2], f32)  # col0: box_in half, col1: tot half
            M2D = pool.tile([128, HALF], f32)
            nc.vector.tensor_tensor(
                out=M2D[:].rearrange("p (h w) -> p h w", w=64),
                in0=RM[:].unsqueeze(2).to_broadcast([128, 32, 64]),
                in1=CM[:].unsqueeze(1).to_broadcast([128, 32, 64]),
                op=Alu.mult)
            MK = pool.tile([128, HALF], f32)
            nc.vector.scalar_tensor_tensor(
                out=MK[:], in0=CAM[:], scalar=1.0, in1=M2D[:],
                op0=Alu.mult, op1=Alu.mult, accum_out=ST[:, 0:1])
            AO = pool.tile([128, HALF], f32)
            nc.scalar.activation(
                out=AO[:], in_=CAM[:], func=Act.Copy, accum_out=ST[:, 1:2])

            # ---- per-pair CAM epilogue, computed per HALF on all 128
            # partitions (s_in/s_out/g1/g2 are identical across a pair's two
            # halves since IC is duplicated; the per-half box_in/tot sums add
            # up to the pair sums under the final PE column reduction). The
            # constant g1 term is tagged to half 0 only via the HT column.
            ICN = pool.tile([128, 1], f32)
            nc.vector.scalar_tensor_tensor(
                out=ICN[:], in0=IC[:, 3:4], scalar=1.0,
                in1=IC[:, 1:2], op0=Alu.add, op1=Alu.subtract)
            JCN = pool.tile([128, 1], f32)
            nc.vector.scalar_tensor_tensor(
                out=JCN[:], in0=IC[:, 2:3], scalar=1.0,
                in1=IC[:, 0:1], op0=Alu.add, op1=Alu.subtract)
            JR = pool.tile([128, 1], f32)
            nc.vector.tensor_scalar_max(JR[:], JCN[:], 0.0)
            SIN = pool.tile([128, 1], f32)
            nc.vector.scalar_tensor_tensor(
                out=SIN[:], in0=ICN[:], scalar=0.0,
                in1=JR[:], op0=Alu.max, op1=Alu.mult)
            SOUT = pool.tile([128, 1], f32)
            nc.vector.tensor_scalar(
                out=SOUT[:], in0=SIN[:], scalar1=-1.0,
                scalar2=float(HW), op0=Alu.mult, op1=Alu.add)
            M1 = pool.tile([128, 1], f32)
            nc.vector.tensor_scalar_max(M1[:], SIN[:], 1.0)
            R1 = pool.tile([128, 1], f32)
            nc.vector.reciprocal(R1[:], M1[:])
            CIN = pool.tile([128, 1], f32)
            nc.vector.tensor_tensor(
                out=CIN[:], in0=ST[:, 0:1], in1=R1[:], op=Alu.mult)
            NUM = pool.tile([128, 1], f32)
            nc.vector.tensor_tensor(
                out=NUM[:], in0=ST[:, 1:2], in1=ST[:, 0:1],
                op=Alu.subtract)
            M2 = pool.tile([128, 1], f32)
            nc.vector.tensor_scalar_max(M2[:], SOUT[:], 1.0)
            R2 = pool.tile([128, 1], f32)
            nc.vector.reciprocal(R2[:], M2[:])
            COUT = pool.tile([128, 1], f32)
            nc.vector.tensor_tensor(
                out=COUT[:], in0=NUM[:], in1=R2[:], op=Alu.mult)
            G1 = pool.tile([128, 1], f32)
            nc.vector.tensor_scalar(
                out=G1[:], in0=SIN[:], scalar1=0.0, scalar2=None,
                op0=Alu.is_gt)
            G2 = pool.tile([128, 1], f32)
            nc.vector.tensor_scalar(
                out=G2[:], in0=SOUT[:], scalar1=0.0, scalar2=None,
                op0=Alu.is_gt)
            T1 = pool.tile([128, 1], f32)
            nc.vector.tensor_tensor(
                out=T1[:], in0=G1[:], in1=CIN[:], op=Alu.mult)
            T2 = pool.tile([128, 1], f32)
            nc.vector.tensor_tensor(
                out=T2[:], in0=G2[:], in1=COUT[:], op=Alu.mult)
            U = pool.tile([128, 1], f32)
            nc.vector.tensor_tensor(
                out=U[:], in0=G1[:], in1=F[:, _HT:_HT + 1], op=Alu.mult)
            D12 = pool.tile([128, 1], f32)
            nc.vector.tensor_tensor(
                out=D12[:], in0=U[:], in1=T1[:], op=Alu.subtract)
            nc.vector.tensor_tensor(
                out=PART[:, 2:3], in0=D12[:], in1=T2[:], op=Alu.add)

            # ---- L1 loss (ACT: |d*w| with per-partition scale, accum) ----
            D = pool.tile([128, 4], f32)
            nc.vector.tensor_tensor(
                out=D[:], in0=PBOX[:], in1=GBOX[:], op=Alu.subtract)
            DABS = pool.tile([128, 4], f32)
            nc.scalar.activation(
                out=DABS[:], in_=D[:], func=Act.Abs,
                scale=F[:, _PW:_PW + 1], accum_out=PART[:, 0:1])

            # ---- GIoU ----
            MX = pool.tile([128, 4], f32)
            nc.vector.tensor_tensor(
                out=MX[:], in0=PBOX[:], in1=GBOX[:], op=Alu.max)
            MN = pool.tile([128, 4], f32)
            nc.vector.tensor_tensor(
                out=MN[:], in0=PBOX[:], in1=GBOX[:], op=Alu.min)
            IWH = pool.tile([128, 2], f32)
            nc.vector.tensor_tensor(
                out=IWH[:], in0=MN[:, 2:4], in1=MX[:, 0:2], op=Alu.subtract)
            IWHR = pool.tile([128, 2], f32)
            nc.vector.tensor_scalar_max(IWHR[:], IWH[:], 0.0)
            INT = pool.tile([128, 1], f32)
            nc.vector.tensor_tensor(
                out=INT[:], in0=IWHR[:, 0:1], in1=IWHR[:, 1:2], op=Alu.mult)
            EWH = pool.tile([128, 2], f32)
            nc.vector.tensor_tensor(
                out=EWH[:], in0=MX[:, 2:4], in1=MN[:, 0:2], op=Alu.subtract)
            ENC = pool.tile([128, 1], f32)
            nc.vector.tensor_tensor(
                out=ENC[:], in0=EWH[:, 0:1], in1=EWH[:, 1:2], op=Alu.mult)
            DP = pool.tile([128, 2], f32)
            nc.vector.tensor_tensor(
                out=DP[:], in0=PBOX[:, 2:4], in1=PBOX[:, 0:2], op=Alu.subtract)
            A1 = pool.tile([128, 1], f32)
            nc.vector.tensor_tensor(
                out=A1[:], in0=DP[:, 0:1], in1=DP[:, 1:2], op=Alu.mult)
            DG = pool.tile([128, 2], f32)
            nc.vector.tensor_tensor(
                out=DG[:], in0=GBOX[:, 2:4], in1=GBOX[:, 0:2], op=Alu.subtract)
            A2 = pool.tile([128, 1], f32)
            nc.vector.tensor_tensor(
                out=A2[:], in0=DG[:, 0:1], in1=DG[:, 1:2], op=Alu.mult)
            APA = pool.tile([128, 1], f32)
            nc.vector.tensor_tensor(
                out=APA[:], in0=A1[:], in1=A2[:], op=Alu.add)
            UN = pool.tile([128, 1], f32)
            nc.vector.tensor_tensor(
                out=UN[:], in0=APA[:], in1=INT[:], op=Alu.subtract)
            UE = pool.tile([128, 1], f32)
            nc.vector.tensor_scalar_add(UE[:], UN[:], EPS)
            RU = pool.tile([128, 1], f32)
            nc.vector.reciprocal(RU[:], UE[:])
            IOU = pool.tile([128, 1], f32)
            nc.vector.tensor_tensor(
                out=IOU[:], in0=INT[:], in1=RU[:], op=Alu.mult)
            EMU = pool.tile([128, 1], f32)
            nc.vector.tensor_tensor(
                out=EMU[:], in0=ENC[:], in1=UN[:], op=Alu.subtract)
            EE = pool.tile([128, 1], f32)
            nc.vector.tensor_scalar_add(EE[:], ENC[:], EPS)
            RE = pool.tile([128, 1], f32)
            nc.vector.reciprocal(RE[:], EE[:])
            Q = pool.tile([128, 1], f32)
            nc.vector.tensor_tensor(
                out=Q[:], in0=EMU[:], in1=RE[:], op=Alu.mult)
            GIO = pool.tile([128, 1], f32)
            nc.vector.tensor_tensor(
                out=GIO[:], in0=IOU[:], in1=Q[:], op=Alu.subtract)
            nc.vector.tensor_tensor(
                out=PART[:, 1:2], in0=GIO[:], in1=F[:, _PW:_PW + 1],
                op=Alu.mult)

            # ---- cross-partition reduce via PE (partials.T @ ones) ----
            PS = pp.tile([4, 1], f32)
            nc.tensor.matmul(out=PS[:], lhsT=PART[:], rhs=ONES[:],
                             start=True, stop=True)
            OS = pool.tile([4, 1], f32)
            nc.vector.tensor_copy(out=OS[:], in_=PS[:])
            nc.sync.dma_start(out=out.ap(), in_=OS[:])

            if debug:
                nc.sync.dma_start(out=dbg.ap()[:, 0:4], in_=PART[:])
                nc.sync.dma_start(out=dbg.ap()[:, 4:8], in_=IC[:])
                nc.sync.dma_start(out=dbg.ap()[:, 8:10], in_=ST[:])
                nc.sync.dma_start(out=dbg.ap()[:, 10:11], in_=CIN[:])
                nc.sync.dma_start(out=dbg.ap()[:, 11:12], in_=COUT[:])
                nc.sync.dma_start(out=dbg.ap()[:, 12:13], in_=SIN[:])
                nc.sync.dma_start(out=dbg.ap()[:, 13:14], in_=SOUT[:])
                nc.sync.dma_start(out=dbg.ap()[:, 14:15], in_=T1[:])
                nc.sync.dma_start(out=dbg.ap()[:, 15:16], in_=T2[:])
                nc.sync.dma_start(out=dbg.ap()[:, 16:20], in_=PBOX[:])
                nc.sync.dma_start(out=dbg.ap()[:, 20:24], in_=GBOX[:])
                nc.sync.dma_start(out=dbg.ap()[:, 24:25], in_=GIO[:])
                nc.sync.dma_start(out=dbg.ap()[:, 25:26], in_=CIF[:])
                nc.sync.dma_start(out=dbg.ap()[:, 26:27], in_=X3[:])
                nc.sync.dma_start(out=dbg.ap()[:, 27:31], in_=SC[:])
                nc.sync.dma_start(out=dbg.ap()[:, 31:35], in_=MX[:])

    nc.finalize()
    return nc


_NC_CACHE = {}


def _get_nc(debug=False):
    key = bool(debug)
    if key not in _NC_CACHE:
        _NC_CACHE[key] = _build_kernel(debug=debug)
    return _NC_CACHE[key]


def make_in_maps(cam, pred_boxes, gt_boxes, gt_labels, pos_b, pos_class,
                 pos_i, pos_j, pos_gt):
    """Host-side sharding: build the per-core input maps."""
    cam = np.ascontiguousarray(np.asarray(cam, dtype=np.float32))
    pred_boxes = np.ascontiguousarray(np.asarray(pred_boxes, dtype=np.float32))
    gt_boxes = np.ascontiguousarray(np.asarray(gt_boxes, dtype=np.float32))
    gt_labels = np.asarray(gt_labels, dtype=np.int32)
    pos_b = np.asarray(pos_b, dtype=np.int64)
    pos_class = np.asarray(pos_class, dtype=np.int64)
    pos_i = np.asarray(pos_i, dtype=np.int64)
    pos_j = np.asarray(pos_j, dtype=np.int64)
    pos_gt = np.asarray(pos_gt, dtype=np.int64)

    # shared constant columns
    kvals = np.arange(1, 64, dtype=np.float32)                 # [63]
    colvals = np.arange(64, dtype=np.float32)                  # [64]
    rowvals = np.empty((128, 32), dtype=np.float32)
    rowvals[:64] = np.arange(32, dtype=np.float32)
    rowvals[64:] = np.arange(32, 64, dtype=np.float32)
    p = np.arange(128)
    pair = p % 64
    half = p // 64
    baseidx = (2 * C * (pair // K) + half).astype(np.float32)  # 160*b_loc + half

    in_maps = []
    for c in range(NCORES):
        b0 = c * BPC
        cam2 = cam[b0:b0 + BPC].reshape(BPC * C * 2, HALF)
        predc = pred_boxes[b0:b0 + BPC].reshape(BPC * C * H * W, 4)
        gtbc = gt_boxes[b0:b0 + BPC].reshape(PAIRS, 4)
        glabc = gt_labels[b0:b0 + BPC].reshape(PAIRS)

        fpk = np.zeros((128, FCOLS), dtype=np.float32)
        fpk[:, _KV0:_KV0 + 63] = kvals
        fpk[:, _CV0:_CV0 + 64] = colvals
        fpk[:, _RV0:_RV0 + 32] = rowvals
        fpk[:, _GT0:_GT0 + 4] = gtbc[pair]
        fpk[:, _GL] = glabc[pair].astype(np.float32)
        fpk[:, _BI] = baseidx
        fpk[:, _HT] = (p < 64).astype(np.float32)

        sel = (pos_b // BPC) == c
        n = int(sel.sum())
        assert n <= 128, (
            f"core {c} got {n} positives; kernel pos capacity is 128")
        fpk[:n, _PW] = 1.0
        fpk[:n, _PB] = (pos_b[sel] - b0).astype(np.float32)
        fpk[:n, _PC] = pos_class[sel].astype(np.float32)
        fpk[:n, _PI] = pos_i[sel].astype(np.float32)
        fpk[:n, _PJ] = pos_j[sel].astype(np.float32)
        fpk[:n, _PG] = pos_gt[sel].astype(np.float32)

        in_maps.append({
            "cam2": np.ascontiguousarray(cam2),
            "pred": np.ascontiguousarray(predc),
            "gtb": np.ascontiguousarray(gtbc),
            "fpk": fpk,
        })
    return in_maps


def combine_outputs(results):
    """Host-side unshard: add per-core partial sums, apply final scaling."""
    P_total = B * K  # 512 positives and 512 cam terms
    l1_sum = 0.0
    gw_sum = 0.0
    term_sum = 0.0
    for r in results:
        o = np.asarray(r["out"], dtype=np.float64).reshape(4)
        l1_sum += o[0]
        gw_sum += o[1]
        term_sum += o[2]
    loss_l1 = l1_sum / (4.0 * P_total)
    loss_giou = 1.0 - gw_sum / P_total
    loss_cam = term_sum / P_total
    loss_total = (LAMBDA_L1 * loss_l1 + LAMBDA_GIOU * loss_giou
                  + LAMBDA_CAM * loss_cam)
    return np.array([loss_l1, loss_giou, loss_cam, loss_total],
                    dtype=np.float32)


def kernel(cam, pred_boxes, gt_boxes, gt_labels, pos_b, pos_class, pos_i,
           pos_j, pos_gt, _debug=False, _trace=False):
    from concourse.bass_utils import run_bass_kernel_spmd

    nc = _get_nc(debug=_debug)
    in_maps = make_in_maps(cam, pred_boxes, gt_boxes, gt_labels, pos_b,
                           pos_class, pos_i, pos_j, pos_gt)
    res = run_bass_kernel_spmd(nc, in_maps, core_ids=list(range(NCORES)),
                               trace=_trace)
    out = combine_outputs(res.results)
    if _debug or _trace:
        return out, res
    return out


# revision 8
# speedup vs baseline: 1.0399x; 1.0399x over previous
"""Trainium2 Bass kernel for DetectionLoss (L1 + GIoU + CAM supervision).

Contract: kernel(**inputs) takes the FULL unsharded inputs (numpy arrays,
keyed as in setup_inputs()) and returns the FULL output, a float32 [4] array
[loss_l1, loss_giou, loss_cam, loss_total].

Sharding: data-parallel over batch B=16 across 8 NeuronCores (2 batches per
core). Positives (pos_*) are routed host-side to the core that owns their
batch, and gather indices (pure index arithmetic) are precomputed host-side
as part of the routing. Each core gathers only the CAM channel maps and
positive boxes it needs via indirect DMA (~1MB instead of ~13MB per core),
computes partial sums (l1, giou*w, cam-term) on device, and the host adds
the 8 partial vectors and applies the final scaling.
"""

import numpy as np

# Problem constants (hardcoded per the task contract).
B, C, H, W, K = 16, 80, 64, 64, 32
NCORES = 8
BPC = B // NCORES           # batches per core = 2
PAIRS = BPC * K             # CAM (box,channel) pairs per core = 64
HW = H * W                  # 4096
HALF = HW // 2              # 2048 (channel map split into 2 partitions)
LAMBDA_L1, LAMBDA_GIOU, LAMBDA_CAM = 1.0, 2.0, 0.5
EPS = 1e-6

# fpack column layout (single packed f32 [128, FCOLS] input per core)
_KV0 = 0            # kvals: 1..63            -> cols [0, 63)
_CV0 = 63           # colvals: 0..63          -> cols [63, 127)
_RV0 = 127          # rowvals (per half)      -> cols [127, 159)
_GT0 = 159          # gt box coords (dup)     -> cols [159, 163)
_PW = 163           # positive slot weight
_HT = 164           # half tag: 1.0 for partitions < 64, else 0.0
FCOLS = 165

# ipack columns (int32 [128, 3]): gather indices, host-precomputed routing
_I_CAM = 0          # row into cam2 [320, 2048]
_I_PRED = 1         # row into pred [BPC*C*H*W, 4]
_I_GT = 2           # row into gtb [64, 4]
ICOLS = 3


def _build_kernel(debug=False):
    import concourse.bacc as bacc
    import concourse.mybir as mybir
    from concourse import bass
    from concourse.tile import TileContext

    f32 = mybir.dt.float32
    i32 = mybir.dt.int32
    Alu = mybir.AluOpType
    Act = mybir.ActivationFunctionType

    nc = bacc.Bacc("TRN2", target_bir_lowering=False, debug=False,
                   num_devices=NCORES)

    cam2 = nc.dram_tensor("cam2", [BPC * C * 2, HALF], f32, kind="ExternalInput")
    pred = nc.dram_tensor("pred", [BPC * C * H * W, 4], f32, kind="ExternalInput")
    gtb = nc.dram_tensor("gtb", [PAIRS, 4], f32, kind="ExternalInput")
    ipk = nc.dram_tensor("ipk", [128, ICOLS], i32, kind="ExternalInput")
    fpk = nc.dram_tensor("fpk", [128, FCOLS], f32, kind="ExternalInput")
    out = nc.dram_tensor("out", [4, 1], f32, kind="ExternalOutput")
    if debug:
        dbg = nc.dram_tensor("dbg", [128, 48], f32, kind="ExternalOutput")

    with TileContext(nc) as tc:
        with (
            tc.tile_pool(name="pool", bufs=1) as pool,
            tc.tile_pool(name="psum", bufs=1, space="PSUM") as pp,
        ):
            # ---- index load + gathers first: the CAM gather is the long pole
            IP = pool.tile([128, ICOLS], i32)
            nc.sync.dma_start(out=IP[:], in_=ipk.ap())
            CAM = pool.tile([128, HALF], f32)
            nc.gpsimd.indirect_dma_start(
                out=CAM[:], out_offset=None, in_=cam2.ap(),
                in_offset=bass.IndirectOffsetOnAxis(ap=IP[:, _I_CAM:_I_CAM + 1],
                                                    axis=0))
            PG = pool.tile([128, 8], f32)  # cols 0:4 pred box, 4:8 gt box
            nc.gpsimd.indirect_dma_start(
                out=PG[:, 0:4], out_offset=None, in_=pred.ap(),
                in_offset=bass.IndirectOffsetOnAxis(ap=IP[:, _I_PRED:_I_PRED + 1],
                                                    axis=0))
            nc.gpsimd.indirect_dma_start(
                out=PG[:, 4:8], out_offset=None, in_=gtb.ap(),
                in_offset=bass.IndirectOffsetOnAxis(ap=IP[:, _I_GT:_I_GT + 1],
                                                    axis=0))

            F = pool.tile([128, FCOLS], f32)
            nc.sync.dma_start(out=F[:], in_=fpk.ap())

            PART = pool.tile([128, 4], f32)
            nc.gpsimd.memset(PART[:], 0.0)
            ONES = pool.tile([128, 1], f32)
            nc.gpsimd.memset(ONES[:], 1.0)

            # ---- exact floor of 64*coords via comparison-sum ----
            SC = pool.tile([128, 4], f32)
            nc.vector.tensor_scalar_mul(SC[:], F[:, _GT0:_GT0 + 4], float(W))
            GEB = pool.tile([128, 4 * 63], f32)
            nc.vector.tensor_tensor(
                out=GEB[:].rearrange("p (c k) -> p c k", k=63),
                in0=SC[:].unsqueeze(2).to_broadcast([128, 4, 63]),
                in1=F[:, _KV0:_KV0 + 63].unsqueeze(1).to_broadcast([128, 4, 63]),
                op=Alu.is_ge)
            IC = pool.tile([128, 4], f32)  # jmin, imin, jmax, imax
            nc.vector.tensor_reduce(
                out=IC[:], in_=GEB[:].rearrange("p (c k) -> p c k", k=63),
                axis=mybir.AxisListType.X, op=Alu.add)

            # ---- row/col interval masks -> 2D mask ----
            CGE = pool.tile([128, 64], f32)
            nc.vector.tensor_scalar(
                out=CGE[:], in0=F[:, _CV0:_CV0 + 64], scalar1=IC[:, 0:1],
                scalar2=None, op0=Alu.is_ge)
            CM = pool.tile([128, 64], f32)
            nc.vector.scalar_tensor_tensor(
                out=CM[:], in0=F[:, _CV0:_CV0 + 64], scalar=IC[:, 2:3],
                in1=CGE[:], op0=Alu.is_le, op1=Alu.mult)
            RGE = pool.tile([128, 32], f32)
            nc.vector.tensor_scalar(
                out=RGE[:], in0=F[:, _RV0:_RV0 + 32], scalar1=IC[:, 1:2],
                scalar2=None, op0=Alu.is_ge)
            RM = pool.tile([128, 32], f32)
            nc.vector.scalar_tensor_tensor(
                out=RM[:], in0=F[:, _RV0:_RV0 + 32], scalar=IC[:, 3:4],
                in1=RGE[:], op0=Alu.is_le, op1=Alu.mult)
            M2D = pool.tile([128, HALF], f32)
            nc.vector.tensor_tensor(
                out=M2D[:].rearrange("p (h w) -> p h w", w=64),
                in0=RM[:].unsqueeze(2).to_broadcast([128, 32, 64]),
                in1=CM[:].unsqueeze(1).to_broadcast([128, 32, 64]),
                op=Alu.mult)

            # ---- interval-count chain (independent of the CAM data) ----
            ICN = pool.tile([128, 1], f32)
            nc.vector.scalar_tensor_tensor(
                out=ICN[:], in0=IC[:, 3:4], scalar=1.0,
                in1=IC[:, 1:2], op0=Alu.add, op1=Alu.subtract)
            JCN = pool.tile([128, 1], f32)
            nc.vector.scalar_tensor_tensor(
                out=JCN[:], in0=IC[:, 2:3], scalar=1.0,
                in1=IC[:, 0:1], op0=Alu.add, op1=Alu.subtract)
            JR = pool.tile([128, 1], f32)
            nc.vector.tensor_scalar_max(JR[:], JCN[:], 0.0)
            SS = pool.tile([128, 2], f32)  # col0: s_in, col1: s_out
            nc.vector.scalar_tensor_tensor(
                out=SS[:, 0:1], in0=ICN[:], scalar=0.0,
                in1=JR[:], op0=Alu.max, op1=Alu.mult)
            nc.vector.tensor_scalar(
                out=SS[:, 1:2], in0=SS[:, 0:1], scalar1=-1.0,
                scalar2=float(HW), op0=Alu.mult, op1=Alu.add)
            MM = pool.tile([128, 2], f32)
            nc.vector.tensor_scalar_max(MM[:], SS[:], 1.0)
            RR = pool.tile([128, 2], f32)
            nc.vector.reciprocal(RR[:], MM[:])
            G12 = pool.tile([128, 2], f32)
            nc.vector.tensor_scalar(
                out=G12[:], in0=SS[:], scalar1=0.0, scalar2=None,
                op0=Alu.is_gt)

            # ---- GIoU + L1 on gathered positives (independent of CAM) ----
            D = pool.tile([128, 4], f32)
            nc.vector.tensor_tensor(
                out=D[:], in0=PG[:, 0:4], in1=PG[:, 4:8], op=Alu.subtract)
            DABS = pool.tile([128, 4], f32)
            nc.scalar.activation(
                out=DABS[:], in_=D[:], func=Act.Abs,
                scale=F[:, _PW:_PW + 1], accum_out=PART[:, 0:1])

            MX = pool.tile([128, 4], f32)
            nc.vector.tensor_tensor(
                out=MX[:], in0=PG[:, 0:4], in1=PG[:, 4:8], op=Alu.max)
            MN = pool.tile([128, 4], f32)
            nc.vector.tensor_tensor(
                out=MN[:], in0=PG[:, 0:4], in1=PG[:, 4:8], op=Alu.min)
            IWH = pool.tile([128, 2], f32)
            nc.vector.tensor_tensor(
                out=IWH[:], in0=MN[:, 2:4], in1=MX[:, 0:2], op=Alu.subtract)
            EWH = pool.tile([128, 2], f32)
            nc.vector.tensor_tensor(
                out=EWH[:], in0=MX[:, 2:4], in1=MN[:, 0:2], op=Alu.subtract)
            W1 = pool.tile([128, 1], f32)
            nc.vector.tensor_scalar_max(W1[:], IWH[:, 1:2], 0.0)
            INT = pool.tile([128, 1], f32)
            nc.vector.scalar_tensor_tensor(
                out=INT[:], in0=IWH[:, 0:1], scalar=0.0, in1=W1[:],
                op0=Alu.max, op1=Alu.mult)
            ENC = pool.tile([128, 1], f32)
            nc.vector.tensor_tensor(
                out=ENC[:], in0=EWH[:, 0:1], in1=EWH[:, 1:2], op=Alu.mult)
            DWH = pool.tile([128, 4], f32)  # (pw, ph, gw, gh)
            nc.vector.tensor_tensor(
                out=DWH[:].rearrange("p (b c) -> p b c", c=2),
                in0=PG[:].rearrange("p (b c) -> p b c", c=4)[:, :, 2:4],
                in1=PG[:].rearrange("p (b c) -> p b c", c=4)[:, :, 0:2],
                op=Alu.subtract)
            A12 = pool.tile([128, 2], f32)  # (a1, a2)
            nc.vector.tensor_tensor(
                out=A12[:],
                in0=DWH[:].rearrange("p (b c) -> p b c", c=2)[:, :, 0:1],
                in1=DWH[:].rearrange("p (b c) -> p b c", c=2)[:, :, 1:2],
                op=Alu.mult)
            APA = pool.tile([128, 1], f32)
            nc.vector.tensor_tensor(
                out=APA[:], in0=A12[:, 0:1], in1=A12[:, 1:2], op=Alu.add)
            UEE = pool.tile([128, 2], f32)  # (union+eps, enc+eps)
            nc.vector.scalar_tensor_tensor(
                out=UEE[:, 0:1], in0=APA[:], scalar=EPS, in1=INT[:],
                op0=Alu.add, op1=Alu.subtract)
            nc.vector.tensor_scalar_add(UEE[:, 1:2], ENC[:], EPS)
            RUE = pool.tile([128, 2], f32)
            nc.vector.reciprocal(RUE[:], UEE[:])
            IOU = pool.tile([128, 1], f32)
            nc.vector.tensor_tensor(
                out=IOU[:], in0=INT[:], in1=RUE[:, 0:1], op=Alu.mult)
            EMU = pool.tile([128, 1], f32)  # enc - union
            nc.vector.tensor_tensor(
                out=EMU[:], in0=UEE[:, 1:2], in1=UEE[:, 0:1], op=Alu.subtract)
            Q = pool.tile([128, 1], f32)
            nc.vector.tensor_tensor(
                out=Q[:], in0=EMU[:], in1=RUE[:, 1:2], op=Alu.mult)
            GIO = pool.tile([128, 1], f32)
            nc.vector.tensor_tensor(
                out=GIO[:], in0=IOU[:], in1=Q[:], op=Alu.subtract)
            nc.vector.tensor_tensor(
                out=PART[:, 1:2], in0=GIO[:], in1=F[:, _PW:_PW + 1],
                op=Alu.mult)

            # ---- big ops on the gathered CAM halves ----
            ST = pool.tile([128, 2], f32)  # col0: box_in half, col1: tot half
            MK = pool.tile([128, HALF], f32)
            nc.vector.scalar_tensor_tensor(
                out=MK[:], in0=CAM[:], scalar=1.0, in1=M2D[:],
                op0=Alu.mult, op1=Alu.mult, accum_out=ST[:, 0:1])
            AO = pool.tile([128, HALF], f32)
            nc.scalar.activation(
                out=AO[:], in_=CAM[:], func=Act.Copy, accum_out=ST[:, 1:2])

            # ---- per-half CAM epilogue on all 128 partitions ----
            CIN = pool.tile([128, 1], f32)
            nc.vector.tensor_tensor(
                out=CIN[:], in0=ST[:, 0:1], in1=RR[:, 0:1], op=Alu.mult)
            NUM = pool.tile([128, 1], f32)
            nc.vector.tensor_tensor(
                out=NUM[:], in0=ST[:, 1:2], in1=ST[:, 0:1], op=Alu.subtract)
            COUT = pool.tile([128, 1], f32)
            nc.vector.tensor_tensor(
                out=COUT[:], in0=NUM[:], in1=RR[:, 1:2], op=Alu.mult)
            HTC = pool.tile([128, 1], f32)  # htag - cam_in
            nc.vector.tensor_tensor(
                out=HTC[:], in0=F[:, _HT:_HT + 1], in1=CIN[:], op=Alu.subtract)
            D12 = pool.tile([128, 1], f32)  # g1*(htag - cam_in)
            nc.vector.tensor_tensor(
                out=D12[:], in0=G12[:, 0:1], in1=HTC[:], op=Alu.mult)
            T2 = pool.tile([128, 1], f32)
            nc.vector.tensor_tensor(
                out=T2[:], in0=G12[:, 1:2], in1=COUT[:], op=Alu.mult)
            nc.vector.tensor_tensor(
                out=PART[:, 2:3], in0=D12[:], in1=T2[:], op=Alu.add)

            # ---- cross-partition reduce via PE (partials.T @ ones) ----
            PS = pp.tile([4, 1], f32)
            nc.tensor.matmul(out=PS[:], lhsT=PART[:], rhs=ONES[:],
                             start=True, stop=True)
            OS = pool.tile([4, 1], f32)
            nc.vector.tensor_copy(out=OS[:], in_=PS[:])
            nc.sync.dma_start(out=out.ap(), in_=OS[:])

            if debug:
                nc.sync.dma_start(out=dbg.ap()[:, 0:4], in_=PART[:])
                nc.sync.dma_start(out=dbg.ap()[:, 4:8], in_=IC[:])
                nc.sync.dma_start(out=dbg.ap()[:, 8:10], in_=ST[:])
                nc.sync.dma_start(out=dbg.ap()[:, 10:11], in_=CIN[:])
                nc.sync.dma_start(out=dbg.ap()[:, 11:12], in_=COUT[:])
                nc.sync.dma_start(out=dbg.ap()[:, 12:14], in_=SS[:])
                nc.sync.dma_start(out=dbg.ap()[:, 14:22], in_=PG[:])
                nc.sync.dma_start(out=dbg.ap()[:, 22:23], in_=GIO[:])
                nc.sync.dma_start(out=dbg.ap()[:, 23:27], in_=SC[:])
                nc.sync.dma_start(out=dbg.ap()[:, 27:31], in_=MX[:])
                nc.sync.dma_start(out=dbg.ap()[:, 31:33], in_=G12[:])

    nc.finalize()
    return nc


_NC_CACHE = {}


def _get_nc(debug=False):
    key = bool(debug)
    if key not in _NC_CACHE:
        _NC_CACHE[key] = _build_kernel(debug=debug)
    return _NC_CACHE[key]


def make_in_maps(cam, pred_boxes, gt_boxes, gt_labels, pos_b, pos_class,
                 pos_i, pos_j, pos_gt):
    """Host-side sharding: build the per-core input maps."""
    cam = np.ascontiguousarray(np.asarray(cam, dtype=np.float32))
    pred_boxes = np.ascontiguousarray(np.asarray(pred_boxes, dtype=np.float32))
    gt_boxes = np.ascontiguousarray(np.asarray(gt_boxes, dtype=np.float32))
    gt_labels = np.asarray(gt_labels, dtype=np.int64)
    pos_b = np.asarray(pos_b, dtype=np.int64)
    pos_class = np.asarray(pos_class, dtype=np.int64)
    pos_i = np.asarray(pos_i, dtype=np.int64)
    pos_j = np.asarray(pos_j, dtype=np.int64)
    pos_gt = np.asarray(pos_gt, dtype=np.int64)

    # shared constant columns
    kvals = np.arange(1, 64, dtype=np.float32)                 # [63]
    colvals = np.arange(64, dtype=np.float32)                  # [64]
    rowvals = np.empty((128, 32), dtype=np.float32)
    rowvals[:64] = np.arange(32, dtype=np.float32)
    rowvals[64:] = np.arange(32, 64, dtype=np.float32)
    p = np.arange(128)
    pair = p % 64
    half = p // 64

    in_maps = []
    for c in range(NCORES):
        b0 = c * BPC
        cam2 = cam[b0:b0 + BPC].reshape(BPC * C * 2, HALF)
        predc = pred_boxes[b0:b0 + BPC].reshape(BPC * C * H * W, 4)
        gtbc = gt_boxes[b0:b0 + BPC].reshape(PAIRS, 4)
        glabc = gt_labels[b0:b0 + BPC].reshape(PAIRS)

        fpk = np.zeros((128, FCOLS), dtype=np.float32)
        fpk[:, _KV0:_KV0 + 63] = kvals
        fpk[:, _CV0:_CV0 + 64] = colvals
        fpk[:, _RV0:_RV0 + 32] = rowvals
        fpk[:, _GT0:_GT0 + 4] = gtbc[pair]
        fpk[:, _HT] = (p < 64).astype(np.float32)

        ipk = np.zeros((128, ICOLS), dtype=np.int32)
        # CAM channel rows: channel (b_loc*C + label), split into 2 halves
        ipk[:, _I_CAM] = 2 * ((pair // K) * C + glabc[pair]) + half

        sel = (pos_b // BPC) == c
        n = int(sel.sum())
        assert n <= 128, (
            f"core {c} got {n} positives; kernel pos capacity is 128")
        b_loc = pos_b[sel] - b0
        fpk[:n, _PW] = 1.0
        ipk[:n, _I_PRED] = (((b_loc * C + pos_class[sel]) * H + pos_i[sel])
                            * W + pos_j[sel])
        ipk[:n, _I_GT] = b_loc * K + pos_gt[sel]

        in_maps.append({
            "cam2": np.ascontiguousarray(cam2),
            "pred": np.ascontiguousarray(predc),
            "gtb": np.ascontiguousarray(gtbc),
            "ipk": ipk,
            "fpk": fpk,
        })
    return in_maps


def combine_outputs(results):
    """Host-side unshard: add per-core partial sums, apply final scaling."""
    P_total = B * K  # 512 positives and 512 cam terms
    l1_sum = 0.0
    gw_sum = 0.0
    term_sum = 0.0
    for r in results:
        o = np.asarray(r["out"], dtype=np.float64).reshape(4)
        l1_sum += o[0]
        gw_sum += o[1]
        term_sum += o[2]
    loss_l1 = l1_sum / (4.0 * P_total)
    loss_giou = 1.0 - gw_sum / P_total
    loss_cam = term_sum / P_total
    loss_total = (LAMBDA_L1 * loss_l1 + LAMBDA_GIOU * loss_giou
                  + LAMBDA_CAM * loss_cam)
    return np.array([loss_l1, loss_giou, loss_cam, loss_total],
                    dtype=np.float32)


def kernel(cam, pred_boxes, gt_boxes, gt_labels, pos_b, pos_class, pos_i,
           pos_j, pos_gt, _debug=False, _trace=False):
    from concourse.bass_utils import run_bass_kernel_spmd

    nc = _get_nc(debug=_debug)
    in_maps = make_in_maps(cam, pred_boxes, gt_boxes, gt_labels, pos_b,
                           pos_class, pos_i, pos_j, pos_gt)
    res = run_bass_kernel_spmd(nc, in_maps, core_ids=list(range(NCORES)),
                               trace=_trace)
    out = combine_outputs(res.results)
    if _debug or _trace:
        return out, res
    return out
